# revision 2
# baseline (speedup 1.0000x reference)
"""Trainium2 Bass kernel v3 for nn_ConvHead — instruction-lean, verified ops.

Per-core layout: all 64 (b,h) rows at partition 16*b+h of one [128, 2048]
tile (rows 16b+8..16b+16 are zero padding). Whole-tile ops handle stats /
14-round count-bisection / sigmoid / masking in one instruction each.

conv: lhsT packs all 3 taps as 96 output rows (32j+h); one fat ACT evac;
SBUF-DMA partition shifts align the taps; 2 adds + 1 DMA place xi rows.
gate: per-b replicated-comb-weight matmul broadcasts sum_h cw[h]*msum to all
128 partitions; apply multiplies src in place reading gate from PSUM.
conv_b is dropped: BatchNorm over l cancels it exactly.
"""
import numpy as np

import concourse.mybir as mybir
from concourse import bacc
from concourse.tile import TileContext

f32 = mybir.dt.float32
AF = mybir.ActivationFunctionType
OP = mybir.AluOpType

B, C, L = 64, 256, 2048
H, KW = 8, 3
Lp = L - KW + 1          # 2046
NCORES = 8
BLOC = B // NCORES       # 8 batches per core
K_TOP = 64
N_ITERS = 16
EPS = 1e-5

_CACHE = {}


def build():
    nc = bacc.Bacc("TRN2")
    srcT = nc.dram_tensor("srcT", [2, 128, BLOC * L], f32, kind="ExternalInput")
    # wjh[c, 96*cb + 32*j + h] = conv_w[h, cb*128+c, j]
    wjh = nc.dram_tensor("wjh", [128, 192], f32, kind="ExternalInput")
    cwrep = nc.dram_tensor("cwrep", [H, 128], f32, kind="ExternalInput")
    gam = nc.dram_tensor("gam", [128, 1], f32, kind="ExternalInput")
    bet = nc.dram_tensor("bet", [128, 1], f32, kind="ExternalInput")
    cbb = nc.dram_tensor("cbb", [128, 1], f32, kind="ExternalInput")
    outT = nc.dram_tensor("outT", [2, 128, BLOC * L], f32, kind="ExternalOutput")

    with TileContext(nc) as tc:
        with (
            tc.tile_pool(name="par", bufs=1) as par,
            tc.tile_pool(name="srcp", bufs=1) as srcp,
            tc.tile_pool(name="wk", bufs=1) as wk,
            tc.tile_pool(name="tapp", bufs=1) as tapp,
            tc.tile_pool(name="cps", bufs=1, space="PSUM") as cpsp,
            tc.tile_pool(name="gps", bufs=1, space="PSUM") as gpsp,
        ):
            wjh_sb = par.tile([128, 192], f32, tag="wjh")
            nc.sync.dma_start(wjh_sb, wjh[:, :])
            cwrep_sb = par.tile([H, 128], f32, tag="cwrep")
            nc.sync.dma_start(cwrep_sb, cwrep[:, :])
            gam_sb = par.tile([128, 1], f32, tag="gam")
            nc.sync.dma_start(gam_sb, gam[:, :])
            bet_sb = par.tile([128, 1], f32, tag="bet")
            nc.sync.dma_start(bet_sb, bet[:, :])
            cbb_sb = par.tile([128, 1], f32, tag="cbb")
            nc.sync.dma_start(cbb_sb, cbb[:, :])

            srcsb = []
            for cb in range(2):
                t = srcp.tile([128, BLOC * L], f32, tag=f"src{cb}",
                              name=f"src{cb}")
                nc.sync.dma_start(t, srcT[cb])
                srcsb.append(t)

            xr = wk.tile([128, L], f32, tag="xr")        # rows 16b+h
            scratch = wk.tile([128, L], f32, tag="scratch")
            mbuf = wk.tile([128, L + 4], f32, tag="mbuf")  # m at cols 2..2050
            msum = wk.tile([128, L], f32, tag="msum")
            nc.vector.memset(xr, 0.0)
            nc.vector.memset(mbuf, 0.0)

            s = {}
            for name in ("sum", "sumsq", "mu", "t1", "veps", "sd", "istd",
                         "scl", "nscl", "bia", "thr", "w", "cnt", "step",
                         "lof"):
                s[name] = wk.tile([128, 1], f32, tag=f"s_{name}",
                                  name=f"s_{name}")

            cps = cpsp.tile([96, L], f32, tag="cps")
            gps = gpsp.tile([128, L], f32, tag="gps")

            # ---- conv ----
            for b in range(BLOC):
                for ci in range(4):
                    for cb in range(2):
                        nc.tensor.matmul(
                            cps[:, 512 * ci:512 * (ci + 1)],
                            lhsT=wjh_sb[:, 96 * cb:96 * (cb + 1)],
                            rhs=srcsb[cb][:, L * b + 512 * ci:
                                          L * b + 512 * (ci + 1)],
                            start=(cb == 0), stop=(cb == 1))
                taps96 = tapp.tile([96, L], f32, tag="taps96", name="taps96")
                tap1 = tapp.tile([8, L], f32, tag="tap1", name="tap1")
                tap2 = tapp.tile([8, L], f32, tag="tap2", name="tap2")
                tsum = tapp.tile([8, Lp], f32, tag="tsum", name="tsum")
                nc.scalar.activation(taps96, cps, AF.Identity)
                nc.sync.dma_start(tap1, taps96[32:40, :])
                nc.sync.dma_start(tap2, taps96[64:72, :])
                nc.vector.tensor_add(tsum, taps96[0:8, 0:Lp], tap1[:, 1:Lp + 1])
                nc.vector.tensor_add(tsum, tsum, tap2[:, 2:Lp + 2])
                nc.sync.dma_start(xr[16 * b:16 * b + 8, 0:Lp], tsum)

            # ---- stats + bisect init ----
            inv_n = 1.0 / Lp
            nc.scalar.activation(scratch, xr, AF.Identity, accum_out=s["sum"])
            nc.scalar.activation(scratch, xr, AF.Square, accum_out=s["sumsq"])
            nc.vector.tensor_scalar_mul(s["mu"], s["sum"], inv_n)
            nc.vector.tensor_scalar(
                out=s["t1"], in0=s["sumsq"], scalar1=inv_n, scalar2=EPS,
                op0=OP.mult, op1=OP.add)
            nc.vector.scalar_tensor_tensor(
                out=s["veps"], in0=s["mu"], scalar=s["mu"][:, :],
                op0=OP.mult, in1=s["t1"], op1=OP.subtract)
            nc.vector.tensor_scalar_mul(s["veps"], s["veps"], -1.0)
            nc.scalar.activation(s["sd"], s["veps"], AF.Sqrt)
            nc.vector.reciprocal(s["istd"], s["sd"])
            nc.vector.tensor_mul(s["scl"], gam_sb, s["istd"])
            nc.vector.tensor_scalar_mul(s["nscl"], s["scl"], -1.0)
            nc.vector.scalar_tensor_tensor(
                out=s["bia"], in0=s["mu"], scalar=s["nscl"][:, :],
                op0=OP.mult, in1=bet_sb, op1=OP.add)
            nc.vector.scalar_tensor_tensor(
                out=s["thr"], in0=s["sd"], scalar=1.90, op0=OP.mult,
                in1=s["mu"], op1=OP.add)
            nc.vector.tensor_scalar_mul(s["w"], s["sd"], 0.45)

            # ---- bisection ----
            for _ in range(N_ITERS):
                nc.vector.tensor_scalar(
                    out=scratch, in0=xr, scalar1=s["thr"][:, :], scalar2=0.0,
                    op0=OP.is_ge, op1=OP.add, accum_out=s["cnt"])
                nc.vector.scalar_tensor_tensor(
                    out=s["step"], in0=s["cnt"], scalar=float(K_TOP),
                    op0=OP.is_ge, in1=s["w"], op1=OP.mult)
                nc.vector.tensor_scalar_mul(s["w"], s["w"], 0.5)
                nc.vector.scalar_tensor_tensor(
                    out=s["thr"], in0=s["step"], scalar=s["w"][:, :],
                    op0=OP.subtract, in1=s["thr"], op1=OP.add)
            nc.vector.scalar_tensor_tensor(
                out=s["lof"], in0=s["w"], scalar=-1.0, op0=OP.mult,
                in1=s["thr"], op1=OP.add)

            # ---- mask + dilate ----
            nc.scalar.activation(scratch, xr, AF.Sigmoid,
                                 bias=s["bia"][:, :], scale=s["scl"][:, :])
            nc.vector.scalar_tensor_tensor(
                out=mbuf[:, 2:2 + L], in0=xr, scalar=s["lof"][:, :],
                op0=OP.is_ge, in1=scratch, op1=OP.mult)
            nc.vector.tensor_add(msum, mbuf[:, 2:2 + L], mbuf[:, 1:1 + L])
            nc.vector.tensor_add(msum, msum, mbuf[:, 0:L])

            # ---- gate + apply ----
            for b in range(BLOC):
                st = tapp.tile([H, L], f32, tag="st", name="st")
                nc.sync.dma_start(st, msum[16 * b:16 * b + 8, :])
                for ci in range(4):
                    nc.tensor.matmul(
                        gps[:, 512 * ci:512 * (ci + 1)],
                        lhsT=cwrep_sb, rhs=st[:, 512 * ci:512 * (ci + 1)],
                        start=True, stop=True)
                for cb in range(2):
                    nc.vector.tensor_mul(
                        srcsb[cb][:, L * b:L * (b + 1)],
                        srcsb[cb][:, L * b:L * (b + 1)], gps)

            for cb in range(2):
                nc.vector.tensor_scalar(
                    out=srcsb[cb], in0=srcsb[cb], scalar1=cbb_sb[:, :],
                    scalar2=0.0, op0=OP.add, op1=OP.add)
                nc.sync.dma_start(outT[cb], srcsb[cb])

    nc.finalize()
    return nc


def _prep_params(conv_w, bn_gamma, bn_beta, comb_w, comb_b):
    wjh = np.zeros((128, 192), np.float32)
    for cb in range(2):
        for j in range(KW):
            for h in range(H):
                wjh[:, 96 * cb + 32 * j + h] = \
                    conv_w[h, 128 * cb:128 * (cb + 1), j]
    cwrep = np.empty((H, 128), np.float32)
    cwrep[:] = (comb_w / float(KW))[:, None]
    gam = np.ones((128, 1), np.float32)
    bet = np.zeros((128, 1), np.float32)
    for b in range(BLOC):
        for h in range(H):
            gam[16 * b + h, 0] = bn_gamma[h]
            bet[16 * b + h, 0] = bn_beta[h]
    cbb = np.full((128, 1), float(np.asarray(comb_b).reshape(-1)[0]),
                  np.float32)
    return wjh, cwrep, gam, bet, cbb


def kernel(src, conv_w, conv_b, bn_gamma, bn_beta, comb_w, comb_b, k):
    from concourse import bass_utils

    src = np.asarray(src, dtype=np.float32)
    conv_w = np.asarray(conv_w, dtype=np.float32)
    bn_gamma = np.asarray(bn_gamma, dtype=np.float32)
    bn_beta = np.asarray(bn_beta, dtype=np.float32)
    comb_w = np.asarray(comb_w, dtype=np.float32)
    comb_b = np.asarray(comb_b, dtype=np.float32)
    assert int(k) == K_TOP, f"kernel compiled for k={K_TOP}, got {k}"
    assert src.shape == (B, C, L)
    # conv_b is unused: per-sample BatchNorm over l cancels it exactly.

    if "nc" not in _CACHE:
        _CACHE["nc"] = build()
    nc = _CACHE["nc"]

    wjh, cwrep, gam, bet, cbb = _prep_params(
        conv_w, bn_gamma, bn_beta, comb_w, comb_b)
    in_maps = []
    for i in range(NCORES):
        slab = src[i * BLOC:(i + 1) * BLOC]
        srcT = np.ascontiguousarray(
            slab.reshape(BLOC, 2, 128, L).transpose(1, 2, 0, 3)
        ).reshape(2, 128, BLOC * L)
        in_maps.append({"srcT": srcT, "wjh": wjh, "cwrep": cwrep,
                        "gam": gam, "bet": bet, "cbb": cbb})
    res = bass_utils.run_bass_kernel_spmd(nc, in_maps,
                                          core_ids=list(range(NCORES)))
    _CACHE["last_results"] = res
    parts = []
    for i in range(NCORES):
        o = res.results[i]["outT"].reshape(2, 128, BLOC, L)
        parts.append(np.ascontiguousarray(
            o.transpose(2, 0, 1, 3)).reshape(BLOC, C, L))
    return np.concatenate(parts, axis=0)


if __name__ == "__main__":
    import reference
    inputs = {k_: np.asarray(v) for k_, v in reference.setup_inputs().items()}
    o = kernel(**inputs)
    print("kernel ran, out shape", o.shape)
